# revision 16
# baseline (speedup 1.0000x reference)
"""Trainium2 Bass kernel for nn_AdaptiveSystem (two-expert conv routing).

Data-parallel over 8 NeuronCores: each core processes 8 of the 64 images and
computes BOTH experts:
  conv1 (3->64, 3x3 s2) for the two experts fused as one M=128 matmul over a
  host-built K=27 im2col, 4-way row-tiled across the PE array (4 pixel groups
  in partition blocks of 32).
  conv2 (64->128, 3x3 s2) as 9 accumulating taps, experts t/f run concurrently
  in the two 64-row halves of the PE array (tile_position row tiling).
  GAP is fused into the conv2 PSUM drain (ScalarE activation accum_out), the
  FC layer + bias run as tiny matmuls, and the confidence mask + blend run on
  VectorE. Host gathers [8,3] per core (blended logits + mask).
"""
import numpy as np
import ml_dtypes
import concourse.bass as bass
import concourse.mybir as mybir
from concourse.bass_utils import run_bass_kernel_spmd

F32 = mybir.dt.float32
BF16 = mybir.dt.bfloat16
AF = mybir.ActivationFunctionType
ALU = mybir.AluOpType

N_CORES = 8
B = 8                 # images per core
NT = 448              # matmul N-tile (one PSUM bank of f32)
NJ = 7                # N-tiles per group/image (7*448 = 3136)
H1R, H1C = 113, 114   # padded h1 layout (112+1 rows, 112+2 cols)
LOG9 = float(np.log(9.0))


# ---------------------------------------------------------------- host prep

def _prep(inputs):
    x = np.asarray(inputs["x"], np.float32)            # [64,3,224,224]
    xp = np.zeros((64, 3, 225, 225), np.float32)
    xp[:, :, :224, :224] = x

    # im2col: partition 32g + (ky*3+kx)*3 + c ; free = 28 rows x 112 cols of
    # conv1 output band g (rows 28g..28g+28)
    A = np.zeros((64, 128, 3136), np.float32)
    for g in range(4):
        for ky in range(3):
            for kx in range(3):
                for c in range(3):
                    p = 32 * g + (ky * 3 + kx) * 3 + c
                    blk = xp[:, c, 56 * g + ky: 56 * g + ky + 56: 2,
                             kx: kx + 224: 2]          # [64, 28, 112]
                    A[:, p, :] = blk.reshape(64, 3136)
    A = A.astype(ml_dtypes.bfloat16)

    t_w1 = np.asarray(inputs["t_w1"], np.float32)
    f_w1 = np.asarray(inputs["f_w1"], np.float32)
    w1s = np.zeros((128, 128), np.float32)
    for g in range(4):
        for ky in range(3):
            for kx in range(3):
                for c in range(3):
                    p = 32 * g + (ky * 3 + kx) * 3 + c
                    w1s[p, 0:64] = t_w1[:, c, ky, kx]
                    w1s[p, 64:128] = f_w1[:, c, ky, kx]
    w1s = w1s.astype(ml_dtypes.bfloat16)

    t_w2 = np.asarray(inputs["t_w2"], np.float32)      # [128,64,3,3]
    f_w2 = np.asarray(inputs["f_w2"], np.float32)
    w2s = np.zeros((128, 9 * 128), np.float32)
    for T in range(9):
        ky, kx = T // 3, T % 3
        w2s[0:64, 128 * T:128 * T + 128] = t_w2[:, :, ky, kx].T
        w2s[64:128, 128 * T:128 * T + 128] = f_w2[:, :, ky, kx].T
    w2s = w2s.astype(ml_dtypes.bfloat16)

    smalls = np.zeros((128, 12), np.float32)
    smalls[0:64, 0] = np.asarray(inputs["t_b1"], np.float32)
    smalls[64:128, 0] = np.asarray(inputs["f_b1"], np.float32)
    smalls[:, 1] = np.asarray(inputs["t_b2"], np.float32)
    smalls[:, 2] = np.asarray(inputs["f_b2"], np.float32)
    smalls[:, 3:5] = np.asarray(inputs["t_wf"], np.float32) / 3136.0
    smalls[:, 5:7] = np.asarray(inputs["f_wf"], np.float32) / 3136.0
    wfb = np.zeros((128, 4), np.float32)
    wfb[:, 0:2] = np.asarray(inputs["t_wf"], np.float32) / 3136.0
    wfb[:, 2:4] = np.asarray(inputs["f_wf"], np.float32) / 3136.0
    wfb = wfb.astype(ml_dtypes.bfloat16)
    bfb = np.zeros((1, 4), np.float32)
    bfb[0, 0:2] = np.asarray(inputs["t_bf"], np.float32)
    bfb[0, 2:4] = np.asarray(inputs["f_bf"], np.float32)
    bfb = bfb.astype(ml_dtypes.bfloat16)

    in_maps = []
    for core in range(N_CORES):
        in_maps.append({
            "xcol": np.ascontiguousarray(A[core * B:(core + 1) * B]),
            "w1s": w1s, "w2s": w2s, "smalls": smalls,
            "wfb": wfb, "bfb": bfb,
        })
    return in_maps


# ---------------------------------------------------------------- builder

def _build(debug=False, reps=1):
    nc = bass.Bass(target_bir_lowering=False)

    xcol = nc.declare_dram_parameter("xcol", [B, 128, 3136], BF16, isOutput=False)
    w1d = nc.declare_dram_parameter("w1s", [128, 128], BF16, isOutput=False)
    w2d = nc.declare_dram_parameter("w2s", [128, 9 * 128], BF16, isOutput=False)
    smd = nc.declare_dram_parameter("smalls", [128, 12], F32, isOutput=False)
    wfd = nc.declare_dram_parameter("wfb", [128, 4], BF16, isOutput=False)
    bfd = nc.declare_dram_parameter("bfb", [1, 4], BF16, isOutput=False)
    outd = nc.declare_dram_parameter("out", [B, 3], F32, isOutput=True)
    if debug:
        dbg_h1 = nc.declare_dram_parameter("dbg_h1", [128, H1R * H1C], BF16, isOutput=True)
        dbg_g = nc.declare_dram_parameter("dbg_g", [128, 16], BF16, isOutput=True)
        dbg_gap = nc.declare_dram_parameter("dbg_gap", [128, 2 * NJ], F32, isOutput=True)
        dbg_scr = nc.declare_dram_parameter("dbg_scr", [8, 16], F32, isOutput=True)

    from contextlib import ExitStack
    with ExitStack() as ctx:
        x_sb = ctx.enter_context(nc.sbuf_tensor([128, B * 3136], BF16))
        h1a = ctx.enter_context(nc.sbuf_tensor([128, H1R * H1C], BF16))
        h1b = ctx.enter_context(nc.sbuf_tensor([128, H1R * H1C], BF16))
        w1_sb = ctx.enter_context(nc.sbuf_tensor([128, 128], BF16))
        w2_sb = ctx.enter_context(nc.sbuf_tensor([128, 9 * 128], BF16))
        sm_sb = ctx.enter_context(nc.sbuf_tensor([128, 12], F32))
        tmp_sb = ctx.enter_context(nc.sbuf_tensor([128, NT], F32))
        gap_t = ctx.enter_context(nc.sbuf_tensor([128, NJ], F32))
        gap_f = ctx.enter_context(nc.sbuf_tensor([128, NJ], F32))
        g_sb = ctx.enter_context(nc.sbuf_tensor([128, 16], BF16))
        ones_sb = ctx.enter_context(nc.sbuf_tensor([1, 8], BF16))
        wf_sb = ctx.enter_context(nc.sbuf_tensor([128, 4], BF16))
        bf_sb = ctx.enter_context(nc.sbuf_tensor([1, 4], BF16))
        scr_sb = ctx.enter_context(nc.sbuf_tensor([8, 16], F32))
        out_sb = ctx.enter_context(nc.sbuf_tensor([8, 4], F32))
        ps1 = [ctx.enter_context(nc.psum_tensor(f"ps1_{i}", [128, NT], F32)) for i in range(4)]
        ps2t = [ctx.enter_context(nc.psum_tensor(f"ps2t_{i}", [128, NT], F32)) for i in range(2)]
        ps2f = [ctx.enter_context(nc.psum_tensor(f"ps2f_{i}", [128, NT], F32)) for i in range(2)]
        sem_w = ctx.enter_context(nc.semaphore("sem_w"))
        sem_x = ctx.enter_context(nc.semaphore("sem_x"))
        sem_pe = ctx.enter_context(nc.semaphore("sem_pe"))
        sem_act = ctx.enter_context(nc.semaphore("sem_act"))
        sem_dve = ctx.enter_context(nc.semaphore("sem_dve"))
        sem_od = ctx.enter_context(nc.semaphore("sem_od"))
        block = ctx.enter_context(nc.Block())
        h1 = [h1a, h1b]

        def h1v(buf):
            return h1[buf][:].rearrange("p (r c) -> p r c", r=H1R)

        # ---- schedule construction: per-engine op lists with resolved waits
        pe_ops, act_ops, dve_ops, sp_ops = [], [], [], []
        cnt = {"pe": 0, "act": 0, "dve": 0}
        # recorded counter checkpoints
        pe_tick_c1 = {}    # (L,j) -> pe count after conv1 group
        pe_tick_c2 = {}    # (I,j) -> pe count after conv2 group
        drain_c1 = {}      # (L,j) -> (act_cnt, dve_cnt) after conv1 drains
        drain_c2 = {}      # (I,j) -> act_cnt after conv2 t+f drains
        reduce_done = {}   # I -> dve count after its two GAP reduces

        # SP: weights + all image im2col loads (x_sb is never reused)
        def sp_start(sp):
            sp.dma_start(out=w1_sb[:], in_=w1d[:]).then_inc(sem_w, 16)
            sp.dma_start(out=w2_sb[:], in_=w2d[:]).then_inc(sem_w, 16)
            sp.dma_start(out=sm_sb[:], in_=smd[:]).then_inc(sem_w, 16)
            sp.dma_start(out=wf_sb[:], in_=wfd[:]).then_inc(sem_w, 16)
            sp.dma_start(out=bf_sb[:], in_=bfd[:]).then_inc(sem_w, 16)
            for i in range(B):
                sp.dma_start(out=x_sb[:, i * 3136:(i + 1) * 3136],
                             in_=xcol[i]).then_inc(sem_x, 16)
        sp_ops.append(sp_start)

        # DVE init: h1 pad zeroing + ones
        def dve_init(dve):
            nc.vector.memset(ones_sb[:], 1.0)
            for buf in range(2):
                v = h1v(buf)
                nc.vector.memset(v[:, 0:H1R, 112:114], 0.0)
                nc.vector.memset(v[:, 112:113, 0:H1C], 0.0)
            for _ in range(7):
                pass
        dve_ops.append(dve_init)
        cnt["dve"] += 0  # no incs on init ops; ordering via stream position

        first_mm = True
        NL = reps * B
        for L in range(NL + 1):
            I = L - 1
            for j in range(NJ):
                # ---------------- PE: conv1 of image L, tile j
                if L < NL:
                    def pe_c1(pe, L=L, j=j, first=first_mm):
                        if first:
                            pe.wait_ge(sem_w, 80)
                        if j == 0 and L < B:
                            pe.wait_ge(sem_x, 16 * (L + 1))
                        prev = (L, j - 1) if j > 0 else ((L - 1, NJ - 1) if L > 0 else None)
                        if prev is not None:
                            a, d = drain_c1[prev]
                            pe.wait_ge(sem_act, a)
                            pe.wait_ge(sem_dve, d)
                        mm = None
                        for g in range(4):
                            mm = nc.tensor.matmul(
                                ps1[g][:, 0:NT],
                                w1_sb[32 * g:32 * g + 27, :],
                                x_sb[32 * g:32 * g + 27,
                                     (L % B) * 3136 + j * NT: (L % B) * 3136 + (j + 1) * NT],
                                start=True, stop=True,
                                tile_position=(32 * g, 0))
                        mm.then_inc(sem_pe)
                    pe_ops.append(pe_c1)
                    first_mm = False
                    cnt["pe"] += 1
                    pe_tick_c1[(L, j)] = cnt["pe"]

                # ---------------- PE: conv2 of image I, tile j
                if L >= 1:
                    def pe_c2(pe, I=I, j=j):
                        prev = (I, j - 2) if j >= 2 else ((I - 1, NJ - 1 - j) if I > 0 else None)
                        if prev is not None:
                            pe.wait_ge(sem_act, drain_c2[prev])
                        if j == 0:
                            a, d = drain_c1[(I, NJ - 1)]
                            pe.wait_ge(sem_act, a)
                            pe.wait_ge(sem_dve, d)
                        v = h1v(I % 2)
                        pst = ps2t[j % 2][:].rearrange("p (a b) -> p a b", a=8)
                        psf = ps2f[j % 2][:].rearrange("p (a b) -> p a b", a=8)
                        mm = None
                        for T in range(9):
                            ky, kx = T // 3, T % 3
                            rt = v[0:64, 16 * j + ky: 16 * j + ky + 15: 2,
                                   kx: kx + 111: 2]
                            rf = v[64:128, 16 * j + ky: 16 * j + ky + 15: 2,
                                   kx: kx + 111: 2]
                            nc.tensor.matmul(pst, w2_sb[0:64, 128 * T:128 * (T + 1)],
                                             rt, start=(T == 0), stop=(T == 8),
                                             tile_position=(0, 0))
                            mm = nc.tensor.matmul(psf, w2_sb[64:128, 128 * T:128 * (T + 1)],
                                                  rf, start=(T == 0), stop=(T == 8),
                                                  tile_position=(64, 0))
                        mm.then_inc(sem_pe)
                    pe_ops.append(pe_c2)
                    cnt["pe"] += 1
                    pe_tick_c2[(I, j)] = cnt["pe"]

                # ---------------- ACT: conv1 g0 drain (L,j) + conv2 t/f drains (I,j)
                if L < NL:
                    def act_c1(act, L=L, j=j, tick=pe_tick_c1[(L, j)]):
                        act.wait_ge(sem_pe, tick)
                        v = h1v(L % 2)
                        nc.scalar.activation(
                            v[:, 4 * j:4 * j + 4, 0:112],
                            ps1[0][:].rearrange("p (a b) -> p a b", a=4),
                            AF.Relu, bias=sm_sb[:, 0:1]).then_inc(sem_act)
                    act_ops.append(act_c1)
                    cnt["act"] += 1
                a_c1 = cnt["act"]

                if L >= 1:
                    def act_c2(act, I=I, j=j, tick=pe_tick_c2[(I, j)],
                               red=(reduce_done.get(I - 1) if j == 0 else None)):
                        act.wait_ge(sem_pe, tick)
                        if red is not None:
                            act.wait_ge(sem_dve, red)
                        nc.scalar.activation(
                            tmp_sb[:, 0:NT], ps2t[j % 2][:, 0:NT], AF.Relu,
                            bias=sm_sb[:, 1:2],
                            accum_out=gap_t[:, j:j + 1]).then_inc(sem_act)
                        nc.scalar.activation(
                            tmp_sb[:, 0:NT], ps2f[j % 2][:, 0:NT], AF.Relu,
                            bias=sm_sb[:, 2:3],
                            accum_out=gap_f[:, j:j + 1]).then_inc(sem_act)
                    act_ops.append(act_c2)
                    cnt["act"] += 2
                    drain_c2[(I, j)] = cnt["act"]

                # ---------------- DVE: conv1 g1..3 drains (L,j)
                if L < NL:
                    def dve_c1(dve, L=L, j=j, tick=pe_tick_c1[(L, j)]):
                        dve.wait_ge(sem_pe, tick)
                        v = h1v(L % 2)
                        for g in range(1, 4):
                            nc.vector.tensor_scalar(
                                v[:, 28 * g + 4 * j:28 * g + 4 * j + 4, 0:112],
                                ps1[g][:].rearrange("p (a b) -> p a b", a=4),
                                sm_sb[:, 0:1], 0.0,
                                ALU.add, ALU.max).then_inc(sem_dve)
                    dve_ops.append(dve_c1)
                    cnt["dve"] += 3
                drain_c1[(L, j)] = (a_c1, cnt["dve"])

                # ---------------- DVE: GAP reduces after last conv2 tile of I
                if L >= 1 and j == NJ - 1:
                    def dve_red(dve, I=I, need=drain_c2[(I, NJ - 1)]):
                        dve.wait_ge(sem_act, need)
                        with nc.allow_low_precision(reason="bf16 FC inputs; 2e-2 gate"):
                            nc.vector.reduce_sum(g_sb[:, (I % B):(I % B) + 1], gap_t[:, 0:NJ],
                                                 axis=mybir.AxisListType.X).then_inc(sem_dve)
                            nc.vector.reduce_sum(g_sb[:, 8 + (I % B):9 + (I % B)], gap_f[:, 0:NJ],
                                                 axis=mybir.AxisListType.X).then_inc(sem_dve)
                    dve_ops.append(dve_red)
                    cnt["dve"] += 2
                    reduce_done[I] = cnt["dve"]

        # ---------------- FC + blend epilogue
        dve_all = cnt["dve"]
        act_all = cnt["act"]

        def pe_fc(pe):
            pe.wait_ge(sem_dve, dve_all)
            pe.wait_ge(sem_act, act_all)
            nc.tensor.matmul(ps2t[0][0:8, 0:2], g_sb[:, 0:8], wf_sb[:, 0:2],
                             start=True, stop=False, skip_group_check=True)
            nc.tensor.matmul(ps2t[0][0:8, 0:2], ones_sb[0:1, 0:8], bf_sb[0:1, 0:2],
                             start=False, stop=True, skip_group_check=True)
            nc.tensor.matmul(ps2t[0][0:8, 2:4], g_sb[:, 8:16], wf_sb[:, 2:4],
                             start=True, stop=False, skip_group_check=True)
            nc.tensor.matmul(ps2t[0][0:8, 2:4], ones_sb[0:1, 0:8], bf_sb[0:1, 2:4],
                             start=False, stop=True,
                             skip_group_check=True).then_inc(sem_pe)
        pe_ops.append(pe_fc)
        cnt["pe"] += 1
        fc_tick = cnt["pe"]

        def dve_blend(dve, base=cnt["dve"]):
            c = [base]

            def step(inst):
                # same-engine RAW fence: DVE's pipeline does not interlock
                # back-to-back dependent ops
                inst.then_inc(sem_dve)
                c[0] += 1
                dve.wait_ge(sem_dve, c[0])

            dve.wait_ge(sem_pe, fc_tick)
            step(nc.vector.tensor_copy(scr_sb[0:8, 12:16], ps2t[0][0:8, 0:4]))
            lt = scr_sb[0:8, 12:14]
            lf = scr_sb[0:8, 14:16]
            # d = lt0 - lt1 ; |d| ; mask = |d| <= log 9
            step(nc.vector.tensor_tensor(scr_sb[0:8, 0:1], lt[0:8, 0:1],
                                         lt[0:8, 1:2], op=ALU.subtract))
            step(nc.vector.tensor_scalar(scr_sb[0:8, 1:2], scr_sb[0:8, 0:1],
                                         -1.0, None, ALU.mult))
            step(nc.vector.tensor_tensor(scr_sb[0:8, 2:3], scr_sb[0:8, 0:1],
                                         scr_sb[0:8, 1:2], op=ALU.max))
            step(nc.vector.tensor_scalar(out_sb[0:8, 2:3], scr_sb[0:8, 2:3],
                                         LOG9, None, ALU.is_le))
            # blended = lt + 0.3 * mask * (lf - lt)
            step(nc.vector.tensor_tensor(scr_sb[0:8, 4:6], lf, lt,
                                         op=ALU.subtract))
            step(nc.vector.tensor_scalar(scr_sb[0:8, 6:8], scr_sb[0:8, 4:6],
                                         0.3, None, ALU.mult))
            step(nc.vector.tensor_scalar(scr_sb[0:8, 8:10], scr_sb[0:8, 6:8],
                                         out_sb[0:8, 2:3], None, ALU.mult))
            step(nc.vector.tensor_tensor(out_sb[0:8, 0:2], lt,
                                         scr_sb[0:8, 8:10], op=ALU.add))
        dve_ops.append(dve_blend)
        cnt["dve"] += 9
        blend_done = cnt["dve"]

        def sp_end(sp):
            sp.wait_ge(sem_dve, blend_done)
            sp.wait_ge(sem_act, act_all)
            sp.dma_start(out=outd[:], in_=out_sb[0:8, 0:3]).then_inc(sem_od, 16)
            n_od = 16
            if debug:
                sp.dma_start(out=dbg_h1[:], in_=h1[(NL - 1) % 2][:]).then_inc(sem_od, 16)
                sp.dma_start(out=dbg_g[:], in_=g_sb[:]).then_inc(sem_od, 16)
                sp.dma_start(out=dbg_gap[:, 0:NJ], in_=gap_t[:]).then_inc(sem_od, 16)
                sp.dma_start(out=dbg_gap[:, NJ:2 * NJ], in_=gap_f[:]).then_inc(sem_od, 16)
                sp.dma_start(out=dbg_scr[:], in_=scr_sb[:]).then_inc(sem_od, 16)
                n_od += 80
            sp.wait_ge(sem_od, n_od)
            for s in (sem_w, sem_x, sem_pe, sem_act, sem_dve, sem_od):
                sp.sem_clear(s)
        sp_ops.append(sp_end)

        # ---- emit
        @block.sync
        def _(sp):
            for op in sp_ops:
                op(sp)

        @block.tensor
        def _(pe):
            for op in pe_ops:
                op(pe)

        @block.scalar
        def _(act):
            for op in act_ops:
                op(act)

        @block.vector
        def _(dve):
            for op in dve_ops:
                op(dve)

    return nc


_NC = None

def _get_nc():
    global _NC
    if _NC is None:
        _NC = _build()
    return _NC


def _run(inputs, trace=False):
    nc = _get_nc()
    in_maps = _prep(inputs)
    res = run_bass_kernel_spmd(nc, in_maps, core_ids=list(range(N_CORES)),
                               trace=trace)
    outs = np.concatenate([np.asarray(res.results[i]["out"])
                           for i in range(N_CORES)], axis=0)   # [64, 3]
    output = np.ascontiguousarray(outs[:, 0:2].astype(np.float32))
    freq = np.float32(outs[:, 2].mean())
    return output, freq, res


def kernel(**inputs):
    output, freq, _ = _run(inputs, trace=False)
    return output, freq


# revision 17
# speedup vs baseline: 1.0173x; 1.0173x over previous
"""Trainium2 Bass kernel for nn_AdaptiveSystem (two-expert conv routing).

Data-parallel over 8 NeuronCores: each core processes 8 of the 64 images and
computes BOTH experts:
  conv1 (3->64, 3x3 s2) for the two experts fused as one M=128 matmul over a
  host-built K=27 im2col, 4-way row-tiled across the PE array (4 pixel groups
  in partition blocks of 32).
  conv2 (64->128, 3x3 s2) as 9 accumulating taps, experts t/f run concurrently
  in the two 64-row halves of the PE array (tile_position row tiling).
  GAP is fused into the conv2 PSUM drain (ScalarE activation accum_out), the
  FC layer + bias run as tiny matmuls, and the confidence mask + blend run on
  VectorE. Host gathers [8,3] per core (blended logits + mask).
"""
import numpy as np
import ml_dtypes
import concourse.bass as bass
import concourse.mybir as mybir
from concourse.bass_utils import run_bass_kernel_spmd

F32 = mybir.dt.float32
BF16 = mybir.dt.bfloat16
AF = mybir.ActivationFunctionType
ALU = mybir.AluOpType

N_CORES = 8
B = 8                 # images per core
NT = 448              # matmul N-tile (one PSUM bank of f32)
NJ = 7                # N-tiles per group/image (7*448 = 3136)
H1R, H1C = 113, 114   # padded h1 layout (112+1 rows, 112+2 cols)
LOG9 = float(np.log(9.0))


# ---------------------------------------------------------------- host prep

def _prep(inputs):
    x = np.asarray(inputs["x"], np.float32)            # [64,3,224,224]
    xp = np.zeros((64, 3, 225, 225), np.float32)
    xp[:, :, :224, :224] = x

    # im2col: partition 32g + (ky*3+kx)*3 + c ; free = 28 rows x 112 cols of
    # conv1 output band g (rows 28g..28g+28)
    A = np.zeros((64, 128, 3136), np.float32)
    for g in range(4):
        for ky in range(3):
            for kx in range(3):
                for c in range(3):
                    p = 32 * g + (ky * 3 + kx) * 3 + c
                    blk = xp[:, c, 56 * g + ky: 56 * g + ky + 56: 2,
                             kx: kx + 224: 2]          # [64, 28, 112]
                    A[:, p, :] = blk.reshape(64, 3136)
    A = A.astype(ml_dtypes.bfloat16)

    t_w1 = np.asarray(inputs["t_w1"], np.float32)
    f_w1 = np.asarray(inputs["f_w1"], np.float32)
    w1s = np.zeros((128, 128), np.float32)
    for g in range(4):
        for ky in range(3):
            for kx in range(3):
                for c in range(3):
                    p = 32 * g + (ky * 3 + kx) * 3 + c
                    w1s[p, 0:64] = t_w1[:, c, ky, kx]
                    w1s[p, 64:128] = f_w1[:, c, ky, kx]
    w1s = w1s.astype(ml_dtypes.bfloat16)

    t_w2 = np.asarray(inputs["t_w2"], np.float32)      # [128,64,3,3]
    f_w2 = np.asarray(inputs["f_w2"], np.float32)
    w2s = np.zeros((128, 9 * 128), np.float32)
    for T in range(9):
        ky, kx = T // 3, T % 3
        w2s[0:64, 128 * T:128 * T + 128] = t_w2[:, :, ky, kx].T
        w2s[64:128, 128 * T:128 * T + 128] = f_w2[:, :, ky, kx].T
    w2s = w2s.astype(ml_dtypes.bfloat16)

    smalls = np.zeros((128, 12), np.float32)
    smalls[0:64, 0] = np.asarray(inputs["t_b1"], np.float32)
    smalls[64:128, 0] = np.asarray(inputs["f_b1"], np.float32)
    smalls[:, 1] = np.asarray(inputs["t_b2"], np.float32)
    smalls[:, 2] = np.asarray(inputs["f_b2"], np.float32)
    smalls[:, 3:5] = np.asarray(inputs["t_wf"], np.float32) / 3136.0
    smalls[:, 5:7] = np.asarray(inputs["f_wf"], np.float32) / 3136.0
    wf32 = np.zeros((128, 4), np.float32)
    wf32[:, 0:2] = np.asarray(inputs["t_wf"], np.float32) / 3136.0
    wf32[:, 2:4] = np.asarray(inputs["f_wf"], np.float32) / 3136.0
    wf_hi = wf32.astype(ml_dtypes.bfloat16)
    wf_lo = (wf32 - wf_hi.astype(np.float32)).astype(ml_dtypes.bfloat16)
    wfb = np.concatenate([wf_hi, wf_lo], axis=1)      # [128, 8]
    bfb = np.zeros((1, 4), np.float32)
    bfb[0, 0:2] = np.asarray(inputs["t_bf"], np.float32)
    bfb[0, 2:4] = np.asarray(inputs["f_bf"], np.float32)
    bfb = bfb.astype(ml_dtypes.bfloat16)

    in_maps = []
    for core in range(N_CORES):
        in_maps.append({
            "xcol": np.ascontiguousarray(A[core * B:(core + 1) * B]),
            "w1s": w1s, "w2s": w2s, "smalls": smalls,
            "wfb": wfb, "bfb": bfb,
        })
    return in_maps


# ---------------------------------------------------------------- builder

def _build(debug=False, reps=1):
    nc = bass.Bass(target_bir_lowering=False)

    xcol = nc.declare_dram_parameter("xcol", [B, 128, 3136], BF16, isOutput=False)
    w1d = nc.declare_dram_parameter("w1s", [128, 128], BF16, isOutput=False)
    w2d = nc.declare_dram_parameter("w2s", [128, 9 * 128], BF16, isOutput=False)
    smd = nc.declare_dram_parameter("smalls", [128, 12], F32, isOutput=False)
    wfd = nc.declare_dram_parameter("wfb", [128, 8], BF16, isOutput=False)
    bfd = nc.declare_dram_parameter("bfb", [1, 4], BF16, isOutput=False)
    outd = nc.declare_dram_parameter("out", [B, 3], F32, isOutput=True)
    if debug:
        dbg_h1 = nc.declare_dram_parameter("dbg_h1", [128, H1R * H1C], BF16, isOutput=True)
        dbg_g = nc.declare_dram_parameter("dbg_g", [128, 16], BF16, isOutput=True)
        dbg_gap = nc.declare_dram_parameter("dbg_gap", [128, 2 * NJ], F32, isOutput=True)
        dbg_scr = nc.declare_dram_parameter("dbg_scr", [8, 16], F32, isOutput=True)

    from contextlib import ExitStack
    with ExitStack() as ctx:
        x_sb = ctx.enter_context(nc.sbuf_tensor([128, B * 3136], BF16))
        h1a = ctx.enter_context(nc.sbuf_tensor([128, H1R * H1C], BF16))
        h1b = ctx.enter_context(nc.sbuf_tensor([128, H1R * H1C], BF16))
        w1_sb = ctx.enter_context(nc.sbuf_tensor([128, 128], BF16))
        w2_sb = ctx.enter_context(nc.sbuf_tensor([128, 9 * 128], BF16))
        sm_sb = ctx.enter_context(nc.sbuf_tensor([128, 12], F32))
        tmp_sb = ctx.enter_context(nc.sbuf_tensor([128, NT], F32))
        gap_t = ctx.enter_context(nc.sbuf_tensor([128, NJ], F32))
        gap_f = ctx.enter_context(nc.sbuf_tensor([128, NJ], F32))
        g_sb = ctx.enter_context(nc.sbuf_tensor([128, 16], F32))
        g_hi = ctx.enter_context(nc.sbuf_tensor([128, 16], BF16))
        g_lo = ctx.enter_context(nc.sbuf_tensor([128, 16], BF16))
        g_tmp = ctx.enter_context(nc.sbuf_tensor([128, 16], F32))
        ones_sb = ctx.enter_context(nc.sbuf_tensor([1, 8], BF16))
        wf_sb = ctx.enter_context(nc.sbuf_tensor([128, 8], BF16))
        bf_sb = ctx.enter_context(nc.sbuf_tensor([1, 4], BF16))
        scr_sb = ctx.enter_context(nc.sbuf_tensor([8, 16], F32))
        out_sb = ctx.enter_context(nc.sbuf_tensor([8, 4], F32))
        ps1 = [ctx.enter_context(nc.psum_tensor(f"ps1_{i}", [128, NT], F32)) for i in range(4)]
        ps2t = [ctx.enter_context(nc.psum_tensor(f"ps2t_{i}", [128, NT], F32)) for i in range(2)]
        ps2f = [ctx.enter_context(nc.psum_tensor(f"ps2f_{i}", [128, NT], F32)) for i in range(2)]
        sem_w = ctx.enter_context(nc.semaphore("sem_w"))
        sem_x = ctx.enter_context(nc.semaphore("sem_x"))
        sem_pe = ctx.enter_context(nc.semaphore("sem_pe"))
        sem_act = ctx.enter_context(nc.semaphore("sem_act"))
        sem_dve = ctx.enter_context(nc.semaphore("sem_dve"))
        sem_od = ctx.enter_context(nc.semaphore("sem_od"))
        block = ctx.enter_context(nc.Block())
        h1 = [h1a, h1b]

        def h1v(buf):
            return h1[buf][:].rearrange("p (r c) -> p r c", r=H1R)

        # ---- schedule construction: per-engine op lists with resolved waits
        pe_ops, act_ops, dve_ops, sp_ops = [], [], [], []
        cnt = {"pe": 0, "act": 0, "dve": 0}
        # recorded counter checkpoints
        pe_tick_c1 = {}    # (L,j) -> pe count after conv1 group
        pe_tick_c2 = {}    # (I,j) -> pe count after conv2 group
        drain_c1 = {}      # (L,j) -> (act_cnt, dve_cnt) after conv1 drains
        drain_c2 = {}      # (I,j) -> act_cnt after conv2 t+f drains
        reduce_done = {}   # I -> dve count after its two GAP reduces

        # SP: weights + all image im2col loads (x_sb is never reused)
        def sp_start(sp):
            sp.dma_start(out=w1_sb[:], in_=w1d[:]).then_inc(sem_w, 16)
            sp.dma_start(out=w2_sb[:], in_=w2d[:]).then_inc(sem_w, 16)
            sp.dma_start(out=sm_sb[:], in_=smd[:]).then_inc(sem_w, 16)
            sp.dma_start(out=wf_sb[:], in_=wfd[:]).then_inc(sem_w, 16)
            sp.dma_start(out=bf_sb[:], in_=bfd[:]).then_inc(sem_w, 16)
            for i in range(B):
                sp.dma_start(out=x_sb[:, i * 3136:(i + 1) * 3136],
                             in_=xcol[i]).then_inc(sem_x, 16)
        sp_ops.append(sp_start)

        # DVE init: h1 pad zeroing + ones
        def dve_init(dve):
            nc.vector.memset(ones_sb[:], 1.0)
            for buf in range(2):
                v = h1v(buf)
                nc.vector.memset(v[:, 0:H1R, 112:114], 0.0)
                nc.vector.memset(v[:, 112:113, 0:H1C], 0.0)
            for _ in range(7):
                pass
        dve_ops.append(dve_init)
        cnt["dve"] += 0  # no incs on init ops; ordering via stream position

        first_mm = True
        NL = reps * B
        for L in range(NL + 1):
            I = L - 1
            for j in range(NJ):
                # ---------------- PE: conv1 of image L, tile j
                if L < NL:
                    def pe_c1(pe, L=L, j=j, first=first_mm):
                        if first:
                            pe.wait_ge(sem_w, 80)
                        if j == 0 and L < B:
                            pe.wait_ge(sem_x, 16 * (L + 1))
                        prev = (L, j - 1) if j > 0 else ((L - 1, NJ - 1) if L > 0 else None)
                        if prev is not None:
                            a, d = drain_c1[prev]
                            pe.wait_ge(sem_act, a)
                            pe.wait_ge(sem_dve, d)
                        mm = None
                        for g in range(4):
                            mm = nc.tensor.matmul(
                                ps1[g][:, 0:NT],
                                w1_sb[32 * g:32 * g + 27, :],
                                x_sb[32 * g:32 * g + 27,
                                     (L % B) * 3136 + j * NT: (L % B) * 3136 + (j + 1) * NT],
                                start=True, stop=True,
                                tile_position=(32 * g, 0))
                        mm.then_inc(sem_pe)
                    pe_ops.append(pe_c1)
                    first_mm = False
                    cnt["pe"] += 1
                    pe_tick_c1[(L, j)] = cnt["pe"]

                # ---------------- PE: conv2 of image I, tile j
                if L >= 1:
                    def pe_c2(pe, I=I, j=j):
                        prev = (I, j - 2) if j >= 2 else ((I - 1, NJ - 1 - j) if I > 0 else None)
                        if prev is not None:
                            pe.wait_ge(sem_act, drain_c2[prev])
                        if j == 0:
                            a, d = drain_c1[(I, NJ - 1)]
                            pe.wait_ge(sem_act, a)
                            pe.wait_ge(sem_dve, d)
                        v = h1v(I % 2)
                        pst = ps2t[j % 2][:].rearrange("p (a b) -> p a b", a=8)
                        psf = ps2f[j % 2][:].rearrange("p (a b) -> p a b", a=8)
                        mm = None
                        for T in range(9):
                            ky, kx = T // 3, T % 3
                            rt = v[0:64, 16 * j + ky: 16 * j + ky + 15: 2,
                                   kx: kx + 111: 2]
                            rf = v[64:128, 16 * j + ky: 16 * j + ky + 15: 2,
                                   kx: kx + 111: 2]
                            nc.tensor.matmul(pst, w2_sb[0:64, 128 * T:128 * (T + 1)],
                                             rt, start=(T == 0), stop=(T == 8),
                                             tile_position=(0, 0))
                            mm = nc.tensor.matmul(psf, w2_sb[64:128, 128 * T:128 * (T + 1)],
                                                  rf, start=(T == 0), stop=(T == 8),
                                                  tile_position=(64, 0))
                        mm.then_inc(sem_pe)
                    pe_ops.append(pe_c2)
                    cnt["pe"] += 1
                    pe_tick_c2[(I, j)] = cnt["pe"]

                # ---------------- ACT: conv1 g0 drain (L,j) + conv2 t/f drains (I,j)
                if L < NL:
                    def act_c1(act, L=L, j=j, tick=pe_tick_c1[(L, j)]):
                        act.wait_ge(sem_pe, tick)
                        v = h1v(L % 2)
                        nc.scalar.activation(
                            v[:, 4 * j:4 * j + 4, 0:112],
                            ps1[0][:].rearrange("p (a b) -> p a b", a=4),
                            AF.Relu, bias=sm_sb[:, 0:1]).then_inc(sem_act)
                    act_ops.append(act_c1)
                    cnt["act"] += 1
                a_c1 = cnt["act"]

                if L >= 1:
                    def act_c2(act, I=I, j=j, tick=pe_tick_c2[(I, j)],
                               red=(reduce_done.get(I - 1) if j == 0 else None)):
                        act.wait_ge(sem_pe, tick)
                        if red is not None:
                            act.wait_ge(sem_dve, red)
                        nc.scalar.activation(
                            tmp_sb[:, 0:NT], ps2t[j % 2][:, 0:NT], AF.Relu,
                            bias=sm_sb[:, 1:2],
                            accum_out=gap_t[:, j:j + 1]).then_inc(sem_act)
                        nc.scalar.activation(
                            tmp_sb[:, 0:NT], ps2f[j % 2][:, 0:NT], AF.Relu,
                            bias=sm_sb[:, 2:3],
                            accum_out=gap_f[:, j:j + 1]).then_inc(sem_act)
                    act_ops.append(act_c2)
                    cnt["act"] += 2
                    drain_c2[(I, j)] = cnt["act"]

                # ---------------- DVE: conv1 g1..3 drains (L,j)
                if L < NL:
                    def dve_c1(dve, L=L, j=j, tick=pe_tick_c1[(L, j)]):
                        dve.wait_ge(sem_pe, tick)
                        v = h1v(L % 2)
                        for g in range(1, 4):
                            nc.vector.tensor_scalar(
                                v[:, 28 * g + 4 * j:28 * g + 4 * j + 4, 0:112],
                                ps1[g][:].rearrange("p (a b) -> p a b", a=4),
                                sm_sb[:, 0:1], 0.0,
                                ALU.add, ALU.max).then_inc(sem_dve)
                    dve_ops.append(dve_c1)
                    cnt["dve"] += 3
                drain_c1[(L, j)] = (a_c1, cnt["dve"])

                # ---------------- DVE: GAP reduces after last conv2 tile of I
                if L >= 1 and j == NJ - 1:
                    def dve_red(dve, I=I, need=drain_c2[(I, NJ - 1)]):
                        dve.wait_ge(sem_act, need)
                        nc.vector.reduce_sum(g_sb[:, (I % B):(I % B) + 1], gap_t[:, 0:NJ],
                                             axis=mybir.AxisListType.X).then_inc(sem_dve)
                        nc.vector.reduce_sum(g_sb[:, 8 + (I % B):9 + (I % B)], gap_f[:, 0:NJ],
                                             axis=mybir.AxisListType.X).then_inc(sem_dve)
                    dve_ops.append(dve_red)
                    cnt["dve"] += 2
                    reduce_done[I] = cnt["dve"]

        # ---------------- FC + blend epilogue
        def dve_split(dve, base=cnt["dve"]):
            c = [base]

            def step(inst):
                inst.then_inc(sem_dve)
                c[0] += 1
                dve.wait_ge(sem_dve, c[0])

            with nc.allow_low_precision(reason="hi/lo split; lo carries residual"):
                step(nc.vector.tensor_copy(g_hi[:], g_sb[:]))
                step(nc.vector.tensor_tensor(g_tmp[:], g_sb[:], g_hi[:],
                                             op=ALU.subtract))
                step(nc.vector.tensor_copy(g_lo[:], g_tmp[:]))
        dve_ops.append(dve_split)
        cnt["dve"] += 3

        dve_all = cnt["dve"]
        act_all = cnt["act"]

        def pe_fc(pe):
            pe.wait_ge(sem_dve, dve_all)
            pe.wait_ge(sem_act, act_all)
            nc.tensor.matmul(ps2t[0][0:8, 0:2], g_hi[:, 0:8], wf_sb[:, 0:2],
                             start=True, stop=False, skip_group_check=True)
            nc.tensor.matmul(ps2t[0][0:8, 0:2], g_hi[:, 0:8], wf_sb[:, 4:6],
                             start=False, stop=False, skip_group_check=True)
            nc.tensor.matmul(ps2t[0][0:8, 0:2], g_lo[:, 0:8], wf_sb[:, 0:2],
                             start=False, stop=False, skip_group_check=True)
            nc.tensor.matmul(ps2t[0][0:8, 0:2], ones_sb[0:1, 0:8], bf_sb[0:1, 0:2],
                             start=False, stop=True, skip_group_check=True)
            nc.tensor.matmul(ps2t[0][0:8, 2:4], g_hi[:, 8:16], wf_sb[:, 2:4],
                             start=True, stop=False, skip_group_check=True)
            nc.tensor.matmul(ps2t[0][0:8, 2:4], g_hi[:, 8:16], wf_sb[:, 6:8],
                             start=False, stop=False, skip_group_check=True)
            nc.tensor.matmul(ps2t[0][0:8, 2:4], g_lo[:, 8:16], wf_sb[:, 2:4],
                             start=False, stop=False, skip_group_check=True)
            nc.tensor.matmul(ps2t[0][0:8, 2:4], ones_sb[0:1, 0:8], bf_sb[0:1, 2:4],
                             start=False, stop=True,
                             skip_group_check=True).then_inc(sem_pe)
        pe_ops.append(pe_fc)
        cnt["pe"] += 1
        fc_tick = cnt["pe"]

        def dve_blend(dve, base=cnt["dve"]):
            c = [base]

            def step(inst):
                # same-engine RAW fence: DVE's pipeline does not interlock
                # back-to-back dependent ops
                inst.then_inc(sem_dve)
                c[0] += 1
                dve.wait_ge(sem_dve, c[0])

            dve.wait_ge(sem_pe, fc_tick)
            step(nc.vector.tensor_copy(scr_sb[0:8, 12:16], ps2t[0][0:8, 0:4]))
            lt = scr_sb[0:8, 12:14]
            lf = scr_sb[0:8, 14:16]
            # d = lt0 - lt1 ; |d| ; mask = |d| <= log 9
            step(nc.vector.tensor_tensor(scr_sb[0:8, 0:1], lt[0:8, 0:1],
                                         lt[0:8, 1:2], op=ALU.subtract))
            step(nc.vector.tensor_scalar(scr_sb[0:8, 1:2], scr_sb[0:8, 0:1],
                                         -1.0, None, ALU.mult))
            step(nc.vector.tensor_tensor(scr_sb[0:8, 2:3], scr_sb[0:8, 0:1],
                                         scr_sb[0:8, 1:2], op=ALU.max))
            step(nc.vector.tensor_scalar(out_sb[0:8, 2:3], scr_sb[0:8, 2:3],
                                         LOG9, None, ALU.is_le))
            # blended = lt + 0.3 * mask * (lf - lt)
            step(nc.vector.tensor_tensor(scr_sb[0:8, 4:6], lf, lt,
                                         op=ALU.subtract))
            step(nc.vector.tensor_scalar(scr_sb[0:8, 6:8], scr_sb[0:8, 4:6],
                                         0.3, None, ALU.mult))
            step(nc.vector.tensor_scalar(scr_sb[0:8, 8:10], scr_sb[0:8, 6:8],
                                         out_sb[0:8, 2:3], None, ALU.mult))
            step(nc.vector.tensor_tensor(out_sb[0:8, 0:2], lt,
                                         scr_sb[0:8, 8:10], op=ALU.add))
        dve_ops.append(dve_blend)
        cnt["dve"] += 9
        blend_done = cnt["dve"]

        def sp_end(sp):
            sp.wait_ge(sem_dve, blend_done)
            sp.wait_ge(sem_act, act_all)
            sp.dma_start(out=outd[:], in_=out_sb[0:8, 0:3]).then_inc(sem_od, 16)
            n_od = 16
            if debug:
                sp.dma_start(out=dbg_h1[:], in_=h1[(NL - 1) % 2][:]).then_inc(sem_od, 16)
                sp.dma_start(out=dbg_g[:], in_=g_sb[:]).then_inc(sem_od, 16)
                sp.dma_start(out=dbg_gap[:, 0:NJ], in_=gap_t[:]).then_inc(sem_od, 16)
                sp.dma_start(out=dbg_gap[:, NJ:2 * NJ], in_=gap_f[:]).then_inc(sem_od, 16)
                sp.dma_start(out=dbg_scr[:], in_=scr_sb[:]).then_inc(sem_od, 16)
                n_od += 80
            sp.wait_ge(sem_od, n_od)
            for s in (sem_w, sem_x, sem_pe, sem_act, sem_dve, sem_od):
                sp.sem_clear(s)
        sp_ops.append(sp_end)

        # ---- emit
        @block.sync
        def _(sp):
            for op in sp_ops:
                op(sp)

        @block.tensor
        def _(pe):
            for op in pe_ops:
                op(pe)

        @block.scalar
        def _(act):
            for op in act_ops:
                op(act)

        @block.vector
        def _(dve):
            for op in dve_ops:
                op(dve)

    return nc


_NC = None

def _get_nc():
    global _NC
    if _NC is None:
        _NC = _build()
    return _NC


def _run(inputs, trace=False):
    nc = _get_nc()
    in_maps = _prep(inputs)
    res = run_bass_kernel_spmd(nc, in_maps, core_ids=list(range(N_CORES)),
                               trace=trace)
    outs = np.concatenate([np.asarray(res.results[i]["out"])
                           for i in range(N_CORES)], axis=0)   # [64, 3]
    output = np.ascontiguousarray(outs[:, 0:2].astype(np.float32))
    freq = np.float32(outs[:, 2].mean())
    return output, freq, res


def kernel(**inputs):
    output, freq, _ = _run(inputs, trace=False)
    return output, freq
